# revision 48
# baseline (speedup 1.0000x reference)
"""Trainium2 Bass kernel for the MoE routing layer (nn_MoELayer_20358144983731).

Strategy
--------
Routing depends only on the atom's type (32 types), and with top-2-of-8
routing each atom needs exactly 3 expert MLPs (2 routed + 1 shared) instead
of the reference's dense 9.  The gate is tiny, so it is computed on the host;
atoms are sorted by type and packed into fixed-capacity slots (CAP=2048
atoms; types larger than CAP get a small spill slot), distributed across the
8 NeuronCores.  Every atom of a type shares the same two routed experts and
scalar routing weights, so the whole device program is data-driven (weights /
biases / scales arrive as per-core input tensors) and a single SPMD program
runs on all 8 cores.

Per slot the device computes, transposed (z.T = [dout, atoms]) so the
dout-dim bias lands on partitions:
    y = w0*tanh(X W0 + b0) + w1*tanh(X W1 + b1) + tanh(X Ws + bs)
Matmuls run in bf16 with fp32 PSUM accumulation (weights stationary, atoms
moving, 1024 columns per matmul, one 4-bank PSUM tile per stream, double
buffered); tanh+bias on the scalar engine (one op per stream; the scalar
engine is the throughput-critical engine so nothing else rides its queue);
the 3-stream combine is two fused scalar_tensor_tensor ops on the vector
engine in bf16; results DMA out as bf16 via the gpsimd SWDGE queue (cheap
issue) and are widened to fp32 on the host.
"""

import sys

import numpy as np

try:
    import concourse  # noqa: F401
except ImportError:  # grading container path
    sys.path.insert(0, "/opt/trn_rl_repo")

import ml_dtypes

import concourse.bacc as bacc
import concourse.mybir as mybir
import concourse.tile as tile
from concourse.bass_utils import run_bass_kernel_spmd

NB, NLOC = 4, 16384
DIN, DOUT = 256, 256
NTYPES = 32
N_CORES = 8
NS = 3  # streams: routed expert 0, routed expert 1, shared expert
CAP = 2048  # big-slot capacity (4 PSUM banks at fp32)
BF16 = ml_dtypes.bfloat16
WCOL = NS * 2 * 2 * 128  # weight columns per slot

# --- tuning knobs ---
ASTEP = 512  # moving-operand columns per matmul (walrus ISA cap)
OUT_BF16 = True  # DMA the output in bf16, widen on host

_compiled_cache = {}


def _build_nc(nbig, nspill, sl):
    """Build + compile the SPMD Tile kernel.

    nbig:   number of CAP-length slots per core
    nspill: number of spill slots per core (0 = none)
    sl:     spill slot length (multiple of 128)
    """
    f32 = mybir.dt.float32
    bf16 = mybir.dt.bfloat16
    ydt = bf16 if OUT_BF16 else f32
    Tanh = mybir.ActivationFunctionType.Tanh
    mult = mybir.AluOpType.mult
    add = mybir.AluOpType.add

    nslots = nbig + nspill

    nc = bacc.Bacc("TRN2", target_bir_lowering=False, debug=False)
    xtb_d = nc.dram_tensor("xtb", [nbig * 128, 2 * CAP], bf16, kind="ExternalInput")
    if nspill:
        xts_d = nc.dram_tensor("xts", [nspill * 128, 2 * sl], bf16, kind="ExternalInput")
    w_d = nc.dram_tensor("w", [128, nslots * WCOL], bf16, kind="ExternalInput")
    b_d = nc.dram_tensor("b", [128, nslots * NS * 2], f32, kind="ExternalInput")
    s_d = nc.dram_tensor("s", [128, nslots * 2], f32, kind="ExternalInput")
    yb_d = nc.dram_tensor("yb", [nbig * 2 * 128, CAP], ydt, kind="ExternalOutput")
    if nspill:
        ys_d = nc.dram_tensor("ys", [nspill * 2 * 128, sl], ydt, kind="ExternalOutput")

    with tile.TileContext(nc) as tc:
        with (
            tc.tile_pool(name="const", bufs=1) as constp,
            tc.tile_pool(name="xt", bufs=3) as xtp,
            tc.tile_pool(name="t", bufs=2) as tp,
            tc.tile_pool(name="y", bufs=2) as yp,
            tc.tile_pool(name="ps", bufs=2, space="PSUM") as psp,
        ):
            # a big slot runs first (its first ACT opens the throughput
            # window as early as possible); spill slots fill the middle and
            # the end, where a tiny final slot keeps the drain tail short
            sp = list(range(nbig, nslots))
            if sp:
                slot_order = [0] + sp[:-1] + list(range(1, nbig)) + sp[-1:]
            else:
                slot_order = list(range(nbig))
            first = slot_order[0]

            # first slot gets per-stream weight tiles so the first matmul only
            # waits on one 128KB transfer; other slots use one tile each
            w_first = [
                constp.tile([128, 512], bf16, name=f"wf{s}") for s in range(NS)
            ]
            w_sl = {
                si: constp.tile([128, WCOL], bf16, name=f"w{si}")
                for si in range(nslots)
                if si != first
            }
            # gpsimd DMA queue order tracks who needs what first: the first
            # slot's shared-stream weights gate the very first matmul, the
            # bias table gates the first ACT, the scale table the first
            # combine
            b_sb = constp.tile([128, nslots * NS * 2], f32)
            s_sb = constp.tile([128, nslots * 2], f32)
            # the PE warm-up tile memset must lead the gpsimd queue: the
            # warm-up matmuls below depend on it. Everything on the early
            # critical path (first slot weights, bias) goes through the HWDGE
            # scalar/sync queues - the gpsimd SWDGE path has ~5us delivery
            # latency.
            wz = constp.tile([128, 512], bf16, name="wz")
            nc.gpsimd.memset(wz, 0.0)
            # The head is DMA-bandwidth-bound: only the tensors that gate the
            # first groups ride the low-latency HWDGE queues (scalar/sync);
            # bulk weights stay on the gpsimd SWDGE path, whose ~5us delivery
            # latency conveniently defers their traffic past the critical
            # window. Stream iteration order is (2, 0, 1) below.
            nc.scalar.dma_start(
                out=w_first[2],
                in_=w_d.ap()[:, first * WCOL + 1024 : first * WCOL + 1536],
            )
            nc.sync.dma_start(out=b_sb, in_=b_d.ap())
            xt0 = [
                xtp.tile([128, CAP if first < nbig else sl], bf16,
                         tag=f"xt{k}", name=f"xt{k}")
                for k in range(2)
            ]
            first_src = xtb_d if first < nbig else xts_d
            first_len = CAP if first < nbig else sl
            first_row = (first if first < nbig else first - nbig) * 128
            # first slot's k=0 stream rides the scalar queue (idle until the
            # first ACT) in halves, k=1 the sync queue: the matmul sweep can
            # begin the moment the first k=0 half lands
            fp = max(first_len // 2, 128)
            for k in range(2):
                q = nc.scalar if k == 0 else nc.sync
                for p0 in range(0, first_len, fp):
                    q.dma_start(
                        out=xt0[k][:, p0 : p0 + fp],
                        in_=first_src.ap()[
                            first_row : first_row + 128,
                            k * first_len + p0 : k * first_len + p0 + fp,
                        ],
                    )
            for s in (0, 1):
                nc.scalar.dma_start(
                    out=w_first[s],
                    in_=w_d.ap()[
                        :, first * WCOL + s * 512 : first * WCOL + (s + 1) * 512
                    ],
                )
            nc.gpsimd.dma_start(out=s_sb, in_=s_d.ap())
            # remaining slot weights via the deferred gpsimd path, in
            # processing order
            for si in slot_order[1:]:
                nc.gpsimd.dma_start(
                    out=w_sl[si], in_=w_d.ap()[:, si * WCOL : (si + 1) * WCOL]
                )

            def issue_xt(si, npieces=1):
                big = si < nbig
                slen = CAP if big else sl
                src_d = xtb_d if big else xts_d
                row0 = (si if big else si - nbig) * 128
                tiles = [
                    xtp.tile([128, slen], bf16, tag=f"xt{k}", name=f"xt{k}")
                    for k in range(2)
                ]
                # split large transfers into pieces so early matmuls only wait
                # for the columns they touch (subtile deps); k-major order
                # because the matmul sweep consumes all of k=0 before k=1
                pl = slen // npieces
                for k in range(2):
                    for p in range(npieces):
                        nc.sync.dma_start(
                            out=tiles[k][:, p * pl : (p + 1) * pl],
                            in_=src_d.ap()[
                                row0 : row0 + 128,
                                k * slen + p * pl : k * slen + (p + 1) * pl,
                            ],
                        )
                return tiles

            xt_pending = {first: xt0}
            for nxt in slot_order[1:3]:
                xt_pending[nxt] = issue_xt(nxt, npieces=2 if nxt < nbig else 1)

            # warm-up matmuls on the zero tile: the PE's HAM clock gate only
            # releases (1.2 -> 2.4 GHz) after ~3.4us of sustained activity,
            # so burn that window on dummy work while the first inputs stream
            ps_w = psp.tile([128, CAP], f32, tag="ps", name="ps")
            for wi in range(5):
                nc.tensor.matmul(
                    ps_w[:, :512], wz[:, :128], wz, start=True, stop=True
                )

            for oi, si in enumerate(slot_order):
                big = si < nbig
                slen = CAP if big else sl
                dst_d = yb_d if big else ys_d
                xt_sb = xt_pending.pop(si)
                if oi + 3 < len(slot_order):
                    nxt = slot_order[oi + 3]
                    xt_pending[nxt] = issue_xt(nxt, npieces=2 if nxt < nbig else 1)
                for c in range(2):
                    # unique tag per (slot, c): every t tile is written exactly
                    # once, so Tile never emits buffer-reuse waits on the
                    # (throughput-critical) scalar queue
                    t_sb = tp.tile(
                        [128, NS * slen], bf16, tag=f"t{oi}_{c}", bufs=1, name="t"
                    )
                    # shared stream (s=2) first: the combines need t2+t0 before
                    # t1, so the tail combine only waits on the last stream
                    for s in (2, 0, 1):
                        bcol = (si * NS + s) * 2 + c
                        ps = psp.tile([128, slen], f32, tag="ps", name="ps")
                        for k in range(2):
                            if si == first:
                                lhsT = w_first[s][:, (c * 2 + k) * 128 : (c * 2 + k + 1) * 128]
                            else:
                                blk = (s * 2 + c) * 2 + k
                                lhsT = w_sl[si][:, blk * 128 : (blk + 1) * 128]
                            for a0 in range(0, slen, ASTEP):
                                al = min(ASTEP, slen - a0)
                                nc.tensor.matmul(
                                    ps[:, a0 : a0 + al],
                                    lhsT,
                                    xt_sb[k][:, a0 : a0 + al],
                                    start=(k == 0),
                                    stop=(k == 1),
                                )
                        # tanh + per-partition bias, PSUM -> SBUF (bf16)
                        nc.scalar.activation(
                            t_sb[:, s * slen : (s + 1) * slen],
                            ps,
                            Tanh,
                            bias=b_sb[:, bcol : bcol + 1],
                            scale=1.0,
                        )
                    yrow = ((si if big else si - nbig) * 2 + c) * 128
                    # the final slots' outputs drain through the sync queue in
                    # quarters: the gpsimd SWDGE drain at kernel end otherwise
                    # waits on a late 256KB transfer
                    late = oi >= len(slot_order) - 2
                    if slen <= 512:
                        pieces = ((0, slen),)
                    elif late:
                        q4 = slen // 4
                        pieces = tuple((j * q4, (j + 1) * q4) for j in range(4))
                    else:
                        h = slen // 2
                        pieces = ((0, h), (h, slen))
                    # combine y = s0*t0 + s1*t1 + t2 via tensor_scalar (4x DVE
                    # perf mode for bf16/SBUF) + tensor_tensor (2x) instead of
                    # scalar_tensor_tensor (1x only). Intermediates u/v/w are
                    # DVE-local rotating tiles (same-queue deps, no
                    # semaphores); only the final y tile is read cross-engine
                    # by the output DMA, and its tag is unique per group so no
                    # reuse waits land on any queue.
                    u = yp.tile([128, slen], ydt, tag="u", name="u")
                    v = yp.tile([128, slen], ydt, tag="v", name="v")
                    w = yp.tile([128, slen], ydt, tag="w", name="w")
                    ya = yp.tile(
                        [128, slen], ydt, tag=f"ya{oi}_{c}", bufs=1, name="ya"
                    )
                    nc.vector.tensor_scalar(
                        u, t_sb[:, :slen], s_sb[:, si * 2 : si * 2 + 1], None, mult
                    )
                    nc.vector.tensor_tensor(
                        v, u, t_sb[:, 2 * slen : 3 * slen], add
                    )
                    nc.vector.tensor_scalar(
                        w,
                        t_sb[:, slen : 2 * slen],
                        s_sb[:, si * 2 + 1 : si * 2 + 2],
                        None,
                        mult,
                    )
                    outq = nc.sync if late else nc.gpsimd
                    for h0, h1 in pieces:
                        nc.vector.tensor_tensor(
                            ya[:, h0:h1], v[:, h0:h1], w[:, h0:h1], add
                        )
                        outq.dma_start(
                            out=dst_d.ap()[yrow : yrow + 128, h0:h1],
                            in_=ya[:, h0:h1],
                        )

    nc.compile()
    return nc


def _host_route(type_embeddings, gate_w):
    """Gate on host: per-type top-2 experts + softmax weights (tiny)."""
    logits = type_embeddings.astype(np.float32) @ gate_w.astype(np.float32)
    top2 = np.argsort(-logits, axis=1, kind="stable")[:, :2]
    tv = np.take_along_axis(logits, top2, axis=1)
    e = np.exp(tv - tv.max(axis=1, keepdims=True))
    wts = e / e.sum(axis=1, keepdims=True)
    return top2, wts


def _xt_layout(buf):
    """[nslots, slen, 256] fp32 -> [nslots*128, 2*slen] bf16 with
    row = slot*128 + p, col = k*slen + a, value = buf[slot, a, k*128+p]."""
    ns, slen, _ = buf.shape
    return np.ascontiguousarray(
        buf.reshape(ns, slen, 2, 128).transpose(0, 3, 2, 1)  # [slot, p, k, a]
    ).reshape(ns * 128, 2 * slen).astype(BF16)


def kernel(x, type_embeddings, atom_types, gate_w, expert_w, expert_b,
           shared_w, shared_b, _trace=False, _trace_kwargs=None):
    x = np.asarray(x, dtype=np.float32)
    type_embeddings = np.asarray(type_embeddings, dtype=np.float32)
    atom_types = np.asarray(atom_types)
    gate_w = np.asarray(gate_w, dtype=np.float32)
    expert_w = np.asarray(expert_w, dtype=np.float32)
    expert_b = np.asarray(expert_b, dtype=np.float32)
    shared_w = np.asarray(shared_w, dtype=np.float32)
    shared_b = np.asarray(shared_b, dtype=np.float32)

    top2, wts = _host_route(type_embeddings, gate_w)

    flat_t = atom_types.reshape(-1).astype(np.int64)
    N = flat_t.size
    order = np.argsort(flat_t, kind="stable")
    counts = np.bincount(flat_t, minlength=NTYPES)
    starts = np.zeros(NTYPES + 1, np.int64)
    starts[1:] = np.cumsum(counts)
    xs = x.reshape(N, DIN)[order]

    # device computes CAP atoms per type; the tiny overflow (~1% of atoms)
    # is computed on the host in fp32, exactly like the reference
    TPC = NTYPES // N_CORES  # big slots per core = 4

    big_buf = np.zeros((N_CORES, TPC, CAP, DIN), np.float32)
    for t in range(NTYPES):
        m = min(int(counts[t]), CAP)
        big_buf[t // TPC, t % TPC, :m] = xs[starts[t] : starts[t] + m]

    spill_out = {}
    for t in range(NTYPES):
        if counts[t] > CAP:
            seg = xs[starts[t] + CAP : starts[t + 1]]
            e0, e1 = top2[t]
            z0 = np.tanh(seg @ expert_w[e0] + expert_b[e0])
            z1 = np.tanh(seg @ expert_w[e1] + expert_b[e1])
            zs = np.tanh(seg @ shared_w[0] + shared_b[0])
            spill_out[t] = wts[t, 0] * z0 + wts[t, 1] * z1 + zs

    in_maps = []
    for core in range(N_CORES):
        sts = [core * TPC + g for g in range(TPC)]
        nslots = len(sts)
        w_sel = np.empty((nslots, NS, DIN, DOUT), np.float32)
        b_sel = np.empty((nslots, NS, DOUT), np.float32)
        s_sel = np.empty((nslots, 2), np.float32)
        for i, t in enumerate(sts):
            e0, e1 = top2[t]
            w_sel[i, 0], w_sel[i, 1], w_sel[i, 2] = (
                expert_w[e0], expert_w[e1], shared_w[0],
            )
            b_sel[i, 0], b_sel[i, 1], b_sel[i, 2] = (
                expert_b[e0], expert_b[e1], shared_b[0],
            )
            s_sel[i] = wts[t]

        wb = (
            w_sel.reshape(nslots, NS, 2, 128, 2, 128)  # [i, s, k, p, c, q]
            .transpose(3, 0, 1, 4, 2, 5)  # [p, i, s, c, k, q]
            .reshape(128, nslots * WCOL)
            .astype(BF16)
        )
        bb = (
            b_sel.reshape(nslots, NS, 2, 128)  # [i, s, c, p]
            .transpose(3, 0, 1, 2)
            .reshape(128, nslots * NS * 2)
            .astype(np.float32)
        )
        sb_arr = np.broadcast_to(
            s_sel.reshape(1, nslots * 2), (128, nslots * 2)
        ).astype(np.float32)

        im = {
            "xtb": _xt_layout(big_buf[core]),
            "w": np.ascontiguousarray(wb),
            "b": np.ascontiguousarray(bb),
            "s": np.ascontiguousarray(sb_arr),
        }
        in_maps.append(im)

    key = (TPC, 0, 0)
    if key not in _compiled_cache:
        _compiled_cache[key] = _build_nc(TPC, 0, 0)
    nc = _compiled_cache[key]

    kwargs = {}
    if _trace:
        kwargs["trace"] = True
        kwargs.update(_trace_kwargs or {})
    res = run_bass_kernel_spmd(nc, in_maps, core_ids=list(range(N_CORES)), **kwargs)

    # reassemble
    out_sorted = np.empty((N, DOUT), np.float32)
    for core in range(N_CORES):
        yb = res.results[core]["yb"].astype(np.float32).reshape(TPC, 2, 128, CAP)
        for g in range(TPC):
            t = core * TPC + g
            m = min(int(counts[t]), CAP)
            # [c, p, a] -> [a, c*128+p]
            blk = yb[g, :, :, :m].reshape(256, m).T
            out_sorted[starts[t] : starts[t] + m] = blk
    for t, seg_out in spill_out.items():
        out_sorted[starts[t] + CAP : starts[t + 1]] = seg_out
    out = np.zeros((N, DOUT), np.float32)
    out[order] = out_sorted
    out = out.reshape(NB, NLOC, DOUT)

    if _trace:
        return out, res
    return out


# revision 53
# speedup vs baseline: 1.0666x; 1.0666x over previous
"""Trainium2 Bass kernel for the MoE routing layer (nn_MoELayer_20358144983731).

Strategy
--------
Routing depends only on the atom's type (32 types), and with top-2-of-8
routing each atom needs exactly 3 expert MLPs (2 routed + 1 shared) instead
of the reference's dense 9.  The gate is tiny, so it is computed on the host;
atoms are sorted by type and packed into fixed-capacity slots (CAP=2048
atoms; types larger than CAP get a small spill slot), distributed across the
8 NeuronCores.  Every atom of a type shares the same two routed experts and
scalar routing weights, so the whole device program is data-driven (weights /
biases / scales arrive as per-core input tensors) and a single SPMD program
runs on all 8 cores.

Per slot the device computes, transposed (z.T = [dout, atoms]) so the
dout-dim bias lands on partitions:
    y = w0*tanh(X W0 + b0) + w1*tanh(X W1 + b1) + tanh(X Ws + bs)
Matmuls run in bf16 with fp32 PSUM accumulation (weights stationary, atoms
moving, 1024 columns per matmul, one 4-bank PSUM tile per stream, double
buffered); tanh+bias on the scalar engine (one op per stream; the scalar
engine is the throughput-critical engine so nothing else rides its queue);
the 3-stream combine is two fused scalar_tensor_tensor ops on the vector
engine in bf16; results DMA out as bf16 via the gpsimd SWDGE queue (cheap
issue) and are widened to fp32 on the host.
"""

import sys

import numpy as np

try:
    import concourse  # noqa: F401
except ImportError:  # grading container path
    sys.path.insert(0, "/opt/trn_rl_repo")

import ml_dtypes

import concourse.bacc as bacc
import concourse.mybir as mybir
import concourse.tile as tile
from concourse.bass_utils import run_bass_kernel_spmd

NB, NLOC = 4, 16384
DIN, DOUT = 256, 256
NTYPES = 32
N_CORES = 8
NS = 3  # streams: routed expert 0, routed expert 1, shared expert
CAP = 2048  # big-slot capacity (4 PSUM banks at fp32)
BF16 = ml_dtypes.bfloat16
WCOL = NS * 2 * 2 * 128  # weight columns per slot

# --- tuning knobs ---
ASTEP = 512  # moving-operand columns per matmul (walrus ISA cap)
OUT_BF16 = True  # DMA the output in bf16, widen on host

_compiled_cache = {}


def _build_nc(nbig, nspill, sl):
    """Build + compile the SPMD Tile kernel.

    nbig:   number of CAP-length slots per core
    nspill: number of spill slots per core (0 = none)
    sl:     spill slot length (multiple of 128)
    """
    f32 = mybir.dt.float32
    bf16 = mybir.dt.bfloat16
    ydt = bf16 if OUT_BF16 else f32
    Tanh = mybir.ActivationFunctionType.Tanh
    mult = mybir.AluOpType.mult
    add = mybir.AluOpType.add

    nslots = nbig + nspill

    nc = bacc.Bacc("TRN2", target_bir_lowering=False, debug=False)
    xtb_d = nc.dram_tensor("xtb", [nbig * 128, 2 * CAP], bf16, kind="ExternalInput")
    if nspill:
        xts_d = nc.dram_tensor("xts", [nspill * 128, 2 * sl], bf16, kind="ExternalInput")
    w_d = nc.dram_tensor("w", [128, nslots * WCOL], bf16, kind="ExternalInput")
    b_d = nc.dram_tensor("b", [128, nslots * NS * 2], f32, kind="ExternalInput")
    s_d = nc.dram_tensor("s", [128, nslots * 2], f32, kind="ExternalInput")
    yb_d = nc.dram_tensor("yb", [nbig * 2 * 128, CAP], ydt, kind="ExternalOutput")
    if nspill:
        ys_d = nc.dram_tensor("ys", [nspill * 2 * 128, sl], ydt, kind="ExternalOutput")

    with tile.TileContext(nc) as tc:
        with (
            tc.tile_pool(name="const", bufs=1) as constp,
            tc.tile_pool(name="xt", bufs=3) as xtp,
            tc.tile_pool(name="t", bufs=2) as tp,
            tc.tile_pool(name="y", bufs=2) as yp,
            tc.tile_pool(name="ps", bufs=2, space="PSUM") as psp,
        ):
            # a big slot runs first (its first ACT opens the throughput
            # window as early as possible); spill slots fill the middle and
            # the end, where a tiny final slot keeps the drain tail short
            sp = list(range(nbig, nslots))
            if sp:
                slot_order = [0] + sp[:-1] + list(range(1, nbig)) + sp[-1:]
            else:
                slot_order = list(range(nbig))
            first = slot_order[0]

            # first slot gets per-stream weight tiles so the first matmul only
            # waits on one 128KB transfer; other slots use one tile each
            w_first = [
                constp.tile([128, 512], bf16, name=f"wf{s}") for s in range(NS)
            ]
            w_sl = {
                si: constp.tile([128, WCOL], bf16, name=f"w{si}")
                for si in range(nslots)
                if si != first
            }
            # gpsimd DMA queue order tracks who needs what first: the first
            # slot's shared-stream weights gate the very first matmul, the
            # bias table gates the first ACT, the scale table the first
            # combine
            b_sb = constp.tile([128, nslots * NS * 2], f32)
            s_sb = constp.tile([128, nslots * 2], f32)
            # the PE warm-up tile memset must lead the gpsimd queue: the
            # warm-up matmuls below depend on it. Everything on the early
            # critical path (first slot weights, bias) goes through the HWDGE
            # scalar/sync queues - the gpsimd SWDGE path has ~5us delivery
            # latency.
            wz = constp.tile([128, 512], bf16, name="wz")
            nc.gpsimd.memset(wz, 0.0)
            # The head is DMA-bandwidth-bound: only the tensors that gate the
            # first groups ride the low-latency HWDGE queues (scalar/sync);
            # bulk weights stay on the gpsimd SWDGE path, whose ~5us delivery
            # latency conveniently defers their traffic past the critical
            # window. Stream iteration order is (2, 0, 1) below.
            nc.gpsimd.dma_start(
                out=w_first[2],
                in_=w_d.ap()[:, first * WCOL + 1024 : first * WCOL + 1536],
            )
            nc.gpsimd.dma_start(out=b_sb, in_=b_d.ap())
            xt0 = [
                xtp.tile([128, CAP if first < nbig else sl], bf16,
                         tag=f"xt{k}", name=f"xt{k}")
                for k in range(2)
            ]
            first_src = xtb_d if first < nbig else xts_d
            first_len = CAP if first < nbig else sl
            first_row = (first if first < nbig else first - nbig) * 128
            # first slot's k=0 stream rides the scalar queue (idle until the
            # first ACT) in halves, k=1 the sync queue: the matmul sweep can
            # begin the moment the first k=0 half lands
            fp = max(first_len // 2, 128)
            for k in range(2):
                q = nc.scalar if k == 0 else nc.sync
                for p0 in range(0, first_len, fp):
                    q.dma_start(
                        out=xt0[k][:, p0 : p0 + fp],
                        in_=first_src.ap()[
                            first_row : first_row + 128,
                            k * first_len + p0 : k * first_len + p0 + fp,
                        ],
                    )
            for s in (0, 1):
                nc.gpsimd.dma_start(
                    out=w_first[s],
                    in_=w_d.ap()[
                        :, first * WCOL + s * 512 : first * WCOL + (s + 1) * 512
                    ],
                )
            nc.gpsimd.dma_start(out=s_sb, in_=s_d.ap())
            # first big slot's weights on the sync queue in per-stream pieces
            # (shared stream s=2 first; subtile deps unblock its matmuls as
            # soon as that piece lands); remaining slots via gpsimd
            fb = next(si for si in slot_order[1:] if si < nbig)
            for cols in ((1024, 1536), (0, 512), (512, 1024)):
                nc.sync.dma_start(
                    out=w_sl[fb][:, cols[0] : cols[1]],
                    in_=w_d.ap()[:, fb * WCOL + cols[0] : fb * WCOL + cols[1]],
                )
            for si in slot_order[1:]:
                if si == fb:
                    continue
                nc.gpsimd.dma_start(
                    out=w_sl[si], in_=w_d.ap()[:, si * WCOL : (si + 1) * WCOL]
                )

            def issue_xt(si, npieces=1):
                big = si < nbig
                slen = CAP if big else sl
                src_d = xtb_d if big else xts_d
                row0 = (si if big else si - nbig) * 128
                tiles = [
                    xtp.tile([128, slen], bf16, tag=f"xt{k}", name=f"xt{k}")
                    for k in range(2)
                ]
                # split large transfers into pieces so early matmuls only wait
                # for the columns they touch (subtile deps); k-major order
                # because the matmul sweep consumes all of k=0 before k=1
                pl = slen // npieces
                for k in range(2):
                    for p in range(npieces):
                        nc.sync.dma_start(
                            out=tiles[k][:, p * pl : (p + 1) * pl],
                            in_=src_d.ap()[
                                row0 : row0 + 128,
                                k * slen + p * pl : k * slen + (p + 1) * pl,
                            ],
                        )
                return tiles

            xt_pending = {first: xt0}
            for nxt in slot_order[1:3]:
                xt_pending[nxt] = issue_xt(nxt, npieces=2 if nxt < nbig else 1)

            # warm-up matmuls on the zero tile: the PE's HAM clock gate only
            # releases (1.2 -> 2.4 GHz) after ~3.4us of sustained activity,
            # so burn that window on dummy work while the first inputs stream
            # enough warm-up matmuls to keep the PE continuously busy until
            # the first slot's inputs land (~13us; head DMA is
            # bandwidth-bound) - the first real group then runs at the warm
            # 2.4 GHz clock instead of cold 1.2 GHz
            ps_w = psp.tile([128, CAP], f32, tag="ps", name="ps")
            for wi in range(9):
                nc.tensor.matmul(
                    ps_w[:, :512], wz[:, :128], wz, start=True, stop=True
                )

            for oi, si in enumerate(slot_order):
                big = si < nbig
                slen = CAP if big else sl
                dst_d = yb_d if big else ys_d
                xt_sb = xt_pending.pop(si)
                if oi + 3 < len(slot_order):
                    nxt = slot_order[oi + 3]
                    xt_pending[nxt] = issue_xt(nxt, npieces=2 if nxt < nbig else 1)
                for c in range(2):
                    # unique tag per (slot, c): every t tile is written exactly
                    # once, so Tile never emits buffer-reuse waits on the
                    # (throughput-critical) scalar queue
                    t_sb = tp.tile(
                        [128, NS * slen], bf16, tag=f"t{oi}_{c}", bufs=1, name="t"
                    )
                    # shared stream (s=2) first: the combines need t2+t0 before
                    # t1, so the tail combine only waits on the last stream
                    for s in (2, 0, 1):
                        bcol = (si * NS + s) * 2 + c
                        ps = psp.tile([128, slen], f32, tag="ps", name="ps")
                        for k in range(2):
                            if si == first:
                                lhsT = w_first[s][:, (c * 2 + k) * 128 : (c * 2 + k + 1) * 128]
                            else:
                                blk = (s * 2 + c) * 2 + k
                                lhsT = w_sl[si][:, blk * 128 : (blk + 1) * 128]
                            for a0 in range(0, slen, ASTEP):
                                al = min(ASTEP, slen - a0)
                                nc.tensor.matmul(
                                    ps[:, a0 : a0 + al],
                                    lhsT,
                                    xt_sb[k][:, a0 : a0 + al],
                                    start=(k == 0),
                                    stop=(k == 1),
                                )
                        # tanh + per-partition bias, PSUM -> SBUF (bf16)
                        nc.scalar.activation(
                            t_sb[:, s * slen : (s + 1) * slen],
                            ps,
                            Tanh,
                            bias=b_sb[:, bcol : bcol + 1],
                            scale=1.0,
                        )
                    yrow = ((si if big else si - nbig) * 2 + c) * 128
                    # the final slots' outputs drain through the sync queue in
                    # quarters: the gpsimd SWDGE drain at kernel end otherwise
                    # waits on a late 256KB transfer
                    late = oi >= len(slot_order) - 2
                    if slen <= 512:
                        pieces = ((0, slen),)
                    elif late:
                        q4 = slen // 4
                        pieces = tuple((j * q4, (j + 1) * q4) for j in range(4))
                    else:
                        h = slen // 2
                        pieces = ((0, h), (h, slen))
                    # combine y = s0*t0 + s1*t1 + t2 via tensor_scalar (4x DVE
                    # perf mode for bf16/SBUF) + tensor_tensor (2x) instead of
                    # scalar_tensor_tensor (1x only). Intermediates u/v/w are
                    # DVE-local rotating tiles (same-queue deps, no
                    # semaphores); only the final y tile is read cross-engine
                    # by the output DMA, and its tag is unique per group so no
                    # reuse waits land on any queue.
                    u = yp.tile([128, slen], ydt, tag="u", name="u")
                    v = yp.tile([128, slen], ydt, tag="v", name="v")
                    w = yp.tile([128, slen], ydt, tag="w", name="w")
                    ya = yp.tile(
                        [128, slen], ydt, tag=f"ya{oi}_{c}", bufs=1, name="ya"
                    )
                    nc.vector.tensor_scalar(
                        u, t_sb[:, :slen], s_sb[:, si * 2 : si * 2 + 1], None, mult
                    )
                    nc.vector.tensor_tensor(
                        v, u, t_sb[:, 2 * slen : 3 * slen], add
                    )
                    nc.vector.tensor_scalar(
                        w,
                        t_sb[:, slen : 2 * slen],
                        s_sb[:, si * 2 + 1 : si * 2 + 2],
                        None,
                        mult,
                    )
                    # very last slot alternates sync/scalar for its output
                    # pieces (both free by then) so the issue cost of the
                    # final drains is paid in parallel
                    last_slot = oi == len(slot_order) - 1
                    for pi, (h0, h1) in enumerate(pieces):
                        nc.vector.tensor_tensor(
                            ya[:, h0:h1], v[:, h0:h1], w[:, h0:h1], add
                        )
                        if last_slot:
                            outq = nc.scalar if pi % 2 else nc.sync
                        elif late:
                            outq = nc.sync
                        else:
                            outq = nc.gpsimd
                        outq.dma_start(
                            out=dst_d.ap()[yrow : yrow + 128, h0:h1],
                            in_=ya[:, h0:h1],
                        )

    nc.compile()
    return nc


def _host_route(type_embeddings, gate_w):
    """Gate on host: per-type top-2 experts + softmax weights (tiny)."""
    logits = type_embeddings.astype(np.float32) @ gate_w.astype(np.float32)
    top2 = np.argsort(-logits, axis=1, kind="stable")[:, :2]
    tv = np.take_along_axis(logits, top2, axis=1)
    e = np.exp(tv - tv.max(axis=1, keepdims=True))
    wts = e / e.sum(axis=1, keepdims=True)
    return top2, wts


def _xt_layout(buf):
    """[nslots, slen, 256] fp32 -> [nslots*128, 2*slen] bf16 with
    row = slot*128 + p, col = k*slen + a, value = buf[slot, a, k*128+p]."""
    ns, slen, _ = buf.shape
    return np.ascontiguousarray(
        buf.reshape(ns, slen, 2, 128).transpose(0, 3, 2, 1)  # [slot, p, k, a]
    ).reshape(ns * 128, 2 * slen).astype(BF16)


def kernel(x, type_embeddings, atom_types, gate_w, expert_w, expert_b,
           shared_w, shared_b, _trace=False, _trace_kwargs=None):
    x = np.asarray(x, dtype=np.float32)
    type_embeddings = np.asarray(type_embeddings, dtype=np.float32)
    atom_types = np.asarray(atom_types)
    gate_w = np.asarray(gate_w, dtype=np.float32)
    expert_w = np.asarray(expert_w, dtype=np.float32)
    expert_b = np.asarray(expert_b, dtype=np.float32)
    shared_w = np.asarray(shared_w, dtype=np.float32)
    shared_b = np.asarray(shared_b, dtype=np.float32)

    top2, wts = _host_route(type_embeddings, gate_w)

    flat_t = atom_types.reshape(-1).astype(np.int64)
    N = flat_t.size
    order = np.argsort(flat_t, kind="stable")
    counts = np.bincount(flat_t, minlength=NTYPES)
    starts = np.zeros(NTYPES + 1, np.int64)
    starts[1:] = np.cumsum(counts)
    xs = x.reshape(N, DIN)[order]

    # device computes CAP atoms per type; the tiny overflow (~1% of atoms)
    # is computed on the host in fp32, exactly like the reference
    TPC = NTYPES // N_CORES  # big slots per core = 4

    big_buf = np.zeros((N_CORES, TPC, CAP, DIN), np.float32)
    for t in range(NTYPES):
        m = min(int(counts[t]), CAP)
        big_buf[t // TPC, t % TPC, :m] = xs[starts[t] : starts[t] + m]

    spill_out = {}
    for t in range(NTYPES):
        if counts[t] > CAP:
            seg = xs[starts[t] + CAP : starts[t + 1]]
            e0, e1 = top2[t]
            z0 = np.tanh(seg @ expert_w[e0] + expert_b[e0])
            z1 = np.tanh(seg @ expert_w[e1] + expert_b[e1])
            zs = np.tanh(seg @ shared_w[0] + shared_b[0])
            spill_out[t] = wts[t, 0] * z0 + wts[t, 1] * z1 + zs

    in_maps = []
    for core in range(N_CORES):
        sts = [core * TPC + g for g in range(TPC)]
        nslots = len(sts)
        w_sel = np.empty((nslots, NS, DIN, DOUT), np.float32)
        b_sel = np.empty((nslots, NS, DOUT), np.float32)
        s_sel = np.empty((nslots, 2), np.float32)
        for i, t in enumerate(sts):
            e0, e1 = top2[t]
            w_sel[i, 0], w_sel[i, 1], w_sel[i, 2] = (
                expert_w[e0], expert_w[e1], shared_w[0],
            )
            b_sel[i, 0], b_sel[i, 1], b_sel[i, 2] = (
                expert_b[e0], expert_b[e1], shared_b[0],
            )
            s_sel[i] = wts[t]

        wb = (
            w_sel.reshape(nslots, NS, 2, 128, 2, 128)  # [i, s, k, p, c, q]
            .transpose(3, 0, 1, 4, 2, 5)  # [p, i, s, c, k, q]
            .reshape(128, nslots * WCOL)
            .astype(BF16)
        )
        bb = (
            b_sel.reshape(nslots, NS, 2, 128)  # [i, s, c, p]
            .transpose(3, 0, 1, 2)
            .reshape(128, nslots * NS * 2)
            .astype(np.float32)
        )
        sb_arr = np.broadcast_to(
            s_sel.reshape(1, nslots * 2), (128, nslots * 2)
        ).astype(np.float32)

        im = {
            "xtb": _xt_layout(big_buf[core]),
            "w": np.ascontiguousarray(wb),
            "b": np.ascontiguousarray(bb),
            "s": np.ascontiguousarray(sb_arr),
        }
        in_maps.append(im)

    key = (TPC, 0, 0)
    if key not in _compiled_cache:
        _compiled_cache[key] = _build_nc(TPC, 0, 0)
    nc = _compiled_cache[key]

    kwargs = {}
    if _trace:
        kwargs["trace"] = True
        kwargs.update(_trace_kwargs or {})
    res = run_bass_kernel_spmd(nc, in_maps, core_ids=list(range(N_CORES)), **kwargs)

    # reassemble
    out_sorted = np.empty((N, DOUT), np.float32)
    for core in range(N_CORES):
        yb = res.results[core]["yb"].astype(np.float32).reshape(TPC, 2, 128, CAP)
        for g in range(TPC):
            t = core * TPC + g
            m = min(int(counts[t]), CAP)
            # [c, p, a] -> [a, c*128+p]
            blk = yb[g, :, :, :m].reshape(256, m).T
            out_sorted[starts[t] : starts[t] + m] = blk
    for t, seg_out in spill_out.items():
        out_sorted[starts[t] + CAP : starts[t + 1]] = seg_out
    out = np.zeros((N, DOUT), np.float32)
    out[order] = out_sorted
    out = out.reshape(NB, NLOC, DOUT)

    if _trace:
        return out, res
    return out


# revision 56
# speedup vs baseline: 1.0853x; 1.0175x over previous
"""Trainium2 Bass kernel for the MoE routing layer (nn_MoELayer_20358144983731).

Strategy
--------
Routing depends only on the atom's type (32 types), and with top-2-of-8
routing each atom needs exactly 3 expert MLPs (2 routed + 1 shared) instead
of the reference's dense 9.  The gate is tiny, so it is computed on the host;
atoms are sorted by type and packed into fixed-capacity slots (CAP=2048
atoms; types larger than CAP get a small spill slot), distributed across the
8 NeuronCores.  Every atom of a type shares the same two routed experts and
scalar routing weights, so the whole device program is data-driven (weights /
biases / scales arrive as per-core input tensors) and a single SPMD program
runs on all 8 cores.

Per slot the device computes, transposed (z.T = [dout, atoms]) so the
dout-dim bias lands on partitions:
    y = w0*tanh(X W0 + b0) + w1*tanh(X W1 + b1) + tanh(X Ws + bs)
Matmuls run in bf16 with fp32 PSUM accumulation (weights stationary, atoms
moving, 1024 columns per matmul, one 4-bank PSUM tile per stream, double
buffered); tanh+bias on the scalar engine (one op per stream; the scalar
engine is the throughput-critical engine so nothing else rides its queue);
the 3-stream combine is two fused scalar_tensor_tensor ops on the vector
engine in bf16; results DMA out as bf16 via the gpsimd SWDGE queue (cheap
issue) and are widened to fp32 on the host.
"""

import sys

import numpy as np

try:
    import concourse  # noqa: F401
except ImportError:  # grading container path
    sys.path.insert(0, "/opt/trn_rl_repo")

import ml_dtypes

import concourse.bacc as bacc
import concourse.mybir as mybir
import concourse.tile as tile
from concourse.bass_utils import run_bass_kernel_spmd

NB, NLOC = 4, 16384
DIN, DOUT = 256, 256
NTYPES = 32
N_CORES = 8
NS = 3  # streams: routed expert 0, routed expert 1, shared expert
CAP = 2048  # big-slot capacity (4 PSUM banks at fp32)
BF16 = ml_dtypes.bfloat16
WCOL = NS * 2 * 2 * 128  # weight columns per slot

# --- tuning knobs ---
ASTEP = 512  # moving-operand columns per matmul (walrus ISA cap)
OUT_BF16 = True  # DMA the output in bf16, widen on host

_compiled_cache = {}


def _build_nc(nbig, nspill, sl):
    """Build + compile the SPMD Tile kernel.

    nbig:   number of CAP-length slots per core
    nspill: number of spill slots per core (0 = none)
    sl:     spill slot length (multiple of 128)
    """
    f32 = mybir.dt.float32
    bf16 = mybir.dt.bfloat16
    ydt = bf16 if OUT_BF16 else f32
    Tanh = mybir.ActivationFunctionType.Tanh
    mult = mybir.AluOpType.mult
    add = mybir.AluOpType.add

    nslots = nbig + nspill

    nc = bacc.Bacc("TRN2", target_bir_lowering=False, debug=False)
    xtb_d = nc.dram_tensor("xtb", [nbig * 128, 2 * CAP], bf16, kind="ExternalInput")
    if nspill:
        xts_d = nc.dram_tensor("xts", [nspill * 128, 2 * sl], bf16, kind="ExternalInput")
    w_d = nc.dram_tensor("w", [128, nslots * WCOL], bf16, kind="ExternalInput")
    b_d = nc.dram_tensor("b", [128, nslots * NS * 2], f32, kind="ExternalInput")
    s_d = nc.dram_tensor("s", [128, nslots * 2], f32, kind="ExternalInput")
    yb_d = nc.dram_tensor("yb", [nbig * 2 * 128, CAP], ydt, kind="ExternalOutput")
    if nspill:
        ys_d = nc.dram_tensor("ys", [nspill * 2 * 128, sl], ydt, kind="ExternalOutput")

    with tile.TileContext(nc) as tc:
        with (
            tc.tile_pool(name="const", bufs=1) as constp,
            tc.tile_pool(name="xt", bufs=3) as xtp,
            tc.tile_pool(name="t", bufs=2) as tp,
            tc.tile_pool(name="y", bufs=2) as yp,
            tc.tile_pool(name="ps", bufs=2, space="PSUM") as psp,
        ):
            # a big slot runs first (its first ACT opens the throughput
            # window as early as possible); spill slots fill the middle and
            # the end, where a tiny final slot keeps the drain tail short
            sp = list(range(nbig, nslots))
            if sp:
                slot_order = [0] + sp[:-1] + list(range(1, nbig)) + sp[-1:]
            else:
                slot_order = list(range(nbig))
            first = slot_order[0]

            # first slot gets per-stream weight tiles so the first matmul only
            # waits on one 128KB transfer; other slots use one tile each
            w_first = [
                constp.tile([128, 512], bf16, name=f"wf{s}") for s in range(NS)
            ]
            w_sl = {
                si: constp.tile([128, WCOL], bf16, name=f"w{si}")
                for si in range(nslots)
                if si != first
            }
            # gpsimd DMA queue order tracks who needs what first: the first
            # slot's shared-stream weights gate the very first matmul, the
            # bias table gates the first ACT, the scale table the first
            # combine
            b_sb = constp.tile([128, nslots * NS * 2], f32)
            s_sb = constp.tile([128, nslots * 2], f32)
            # the PE warm-up tile memset must lead the gpsimd queue: the
            # warm-up matmuls below depend on it. Everything on the early
            # critical path (first slot weights, bias) goes through the HWDGE
            # scalar/sync queues - the gpsimd SWDGE path has ~5us delivery
            # latency.
            wz = constp.tile([128, 512], bf16, name="wz")
            nc.gpsimd.memset(wz, 0.0)
            # The head is DMA-bandwidth-bound: only the tensors that gate the
            # first groups ride the low-latency HWDGE queues (scalar/sync);
            # bulk weights stay on the gpsimd SWDGE path, whose ~5us delivery
            # latency conveniently defers their traffic past the critical
            # window. Stream iteration order is (2, 0, 1) below.
            nc.gpsimd.dma_start(
                out=w_first[2],
                in_=w_d.ap()[:, first * WCOL + 1024 : first * WCOL + 1536],
            )
            nc.gpsimd.dma_start(out=b_sb, in_=b_d.ap())
            xt0 = [
                xtp.tile([128, CAP if first < nbig else sl], bf16,
                         tag=f"xt{k}", name=f"xt{k}")
                for k in range(2)
            ]
            first_src = xtb_d if first < nbig else xts_d
            first_len = CAP if first < nbig else sl
            first_row = (first if first < nbig else first - nbig) * 128
            # first slot's k=0 stream rides the scalar queue (idle until the
            # first ACT) in halves, k=1 the sync queue: the matmul sweep can
            # begin the moment the first k=0 half lands
            fp = max(first_len // 2, 128)
            for k in range(2):
                q = nc.scalar if k == 0 else nc.sync
                for p0 in range(0, first_len, fp):
                    q.dma_start(
                        out=xt0[k][:, p0 : p0 + fp],
                        in_=first_src.ap()[
                            first_row : first_row + 128,
                            k * first_len + p0 : k * first_len + p0 + fp,
                        ],
                    )
            for s in (0, 1):
                nc.gpsimd.dma_start(
                    out=w_first[s],
                    in_=w_d.ap()[
                        :, first * WCOL + s * 512 : first * WCOL + (s + 1) * 512
                    ],
                )
            nc.gpsimd.dma_start(out=s_sb, in_=s_d.ap())
            # first big slot's weights on the sync queue in per-stream pieces
            # (shared stream s=2 first; subtile deps unblock its matmuls as
            # soon as that piece lands); remaining slots via gpsimd
            fb = next(si for si in slot_order[1:] if si < nbig)
            for cols in ((1024, 1536), (0, 512), (512, 1024)):
                nc.sync.dma_start(
                    out=w_sl[fb][:, cols[0] : cols[1]],
                    in_=w_d.ap()[:, fb * WCOL + cols[0] : fb * WCOL + cols[1]],
                )
            for si in slot_order[1:]:
                if si == fb:
                    continue
                nc.gpsimd.dma_start(
                    out=w_sl[si], in_=w_d.ap()[:, si * WCOL : (si + 1) * WCOL]
                )

            def issue_xt(si, npieces=1):
                big = si < nbig
                slen = CAP if big else sl
                src_d = xtb_d if big else xts_d
                row0 = (si if big else si - nbig) * 128
                tiles = [
                    xtp.tile([128, slen], bf16, tag=f"xt{k}", name=f"xt{k}")
                    for k in range(2)
                ]
                # prefetched xt rides the gpsimd SWDGE path: its ~5us
                # delivery latency keeps these bulk transfers out of the
                # bandwidth-critical head window, and prefetch runs ~30us
                # ahead of consumption so the latency is free
                pl = slen // npieces
                for k in range(2):
                    for p in range(npieces):
                        nc.gpsimd.dma_start(
                            out=tiles[k][:, p * pl : (p + 1) * pl],
                            in_=src_d.ap()[
                                row0 : row0 + 128,
                                k * slen + p * pl : k * slen + (p + 1) * pl,
                            ],
                        )
                return tiles

            xt_pending = {first: xt0}
            for nxt in slot_order[1:3]:
                xt_pending[nxt] = issue_xt(nxt, npieces=2 if nxt < nbig else 1)

            # warm-up matmuls on the zero tile: the PE's HAM clock gate only
            # releases (1.2 -> 2.4 GHz) after ~3.4us of sustained activity,
            # so burn that window on dummy work while the first inputs stream
            # enough warm-up matmuls to keep the PE continuously busy until
            # the first slot's inputs land (~13us; head DMA is
            # bandwidth-bound) - the first real group then runs at the warm
            # 2.4 GHz clock instead of cold 1.2 GHz
            ps_w = psp.tile([128, CAP], f32, tag="ps", name="ps")
            for wi in range(3):
                nc.tensor.matmul(
                    ps_w[:, :512], wz[:, :128], wz, start=True, stop=True
                )

            for oi, si in enumerate(slot_order):
                big = si < nbig
                slen = CAP if big else sl
                dst_d = yb_d if big else ys_d
                xt_sb = xt_pending.pop(si)
                if oi + 3 < len(slot_order):
                    nxt = slot_order[oi + 3]
                    xt_pending[nxt] = issue_xt(nxt, npieces=2 if nxt < nbig else 1)
                for c in range(2):
                    # unique tag per (slot, c): every t tile is written exactly
                    # once, so Tile never emits buffer-reuse waits on the
                    # (throughput-critical) scalar queue
                    t_sb = tp.tile(
                        [128, NS * slen], bf16, tag=f"t{oi}_{c}", bufs=1, name="t"
                    )
                    # shared stream (s=2) first: the combines need t2+t0 before
                    # t1, so the tail combine only waits on the last stream
                    for s in (2, 0, 1):
                        bcol = (si * NS + s) * 2 + c
                        ps = psp.tile([128, slen], f32, tag="ps", name="ps")
                        for k in range(2):
                            if si == first:
                                lhsT = w_first[s][:, (c * 2 + k) * 128 : (c * 2 + k + 1) * 128]
                            else:
                                blk = (s * 2 + c) * 2 + k
                                lhsT = w_sl[si][:, blk * 128 : (blk + 1) * 128]
                            for a0 in range(0, slen, ASTEP):
                                al = min(ASTEP, slen - a0)
                                nc.tensor.matmul(
                                    ps[:, a0 : a0 + al],
                                    lhsT,
                                    xt_sb[k][:, a0 : a0 + al],
                                    start=(k == 0),
                                    stop=(k == 1),
                                )
                        # tanh + per-partition bias, PSUM -> SBUF (bf16)
                        nc.scalar.activation(
                            t_sb[:, s * slen : (s + 1) * slen],
                            ps,
                            Tanh,
                            bias=b_sb[:, bcol : bcol + 1],
                            scale=1.0,
                        )
                    yrow = ((si if big else si - nbig) * 2 + c) * 128
                    # the final slots' outputs drain through the sync queue in
                    # quarters: the gpsimd SWDGE drain at kernel end otherwise
                    # waits on a late 256KB transfer
                    late = oi >= len(slot_order) - 2
                    if slen <= 512:
                        pieces = ((0, slen),)
                    elif late:
                        q4 = slen // 4
                        pieces = tuple((j * q4, (j + 1) * q4) for j in range(4))
                    else:
                        h = slen // 2
                        pieces = ((0, h), (h, slen))
                    # combine y = s0*t0 + s1*t1 + t2 via tensor_scalar (4x DVE
                    # perf mode for bf16/SBUF) + tensor_tensor (2x) instead of
                    # scalar_tensor_tensor (1x only). Intermediates u/v/w are
                    # DVE-local rotating tiles (same-queue deps, no
                    # semaphores); only the final y tile is read cross-engine
                    # by the output DMA, and its tag is unique per group so no
                    # reuse waits land on any queue.
                    u = yp.tile([128, slen], ydt, tag="u", name="u")
                    v = yp.tile([128, slen], ydt, tag="v", name="v")
                    w = yp.tile([128, slen], ydt, tag="w", name="w")
                    ya = yp.tile(
                        [128, slen], ydt, tag=f"ya{oi}_{c}", bufs=1, name="ya"
                    )
                    nc.vector.tensor_scalar(
                        u, t_sb[:, :slen], s_sb[:, si * 2 : si * 2 + 1], None, mult
                    )
                    nc.vector.tensor_tensor(
                        v, u, t_sb[:, 2 * slen : 3 * slen], add
                    )
                    nc.vector.tensor_scalar(
                        w,
                        t_sb[:, slen : 2 * slen],
                        s_sb[:, si * 2 + 1 : si * 2 + 2],
                        None,
                        mult,
                    )
                    # the very last group alternates sync/scalar for its
                    # output pieces (both free after the final ACT) so the
                    # issue cost of the final drains is paid in parallel
                    last_slot = oi == len(slot_order) - 1 and c == 1
                    for pi, (h0, h1) in enumerate(pieces):
                        nc.vector.tensor_tensor(
                            ya[:, h0:h1], v[:, h0:h1], w[:, h0:h1], add
                        )
                        if last_slot:
                            outq = nc.scalar if pi % 2 else nc.sync
                        elif late:
                            outq = nc.sync
                        else:
                            outq = nc.gpsimd
                        outq.dma_start(
                            out=dst_d.ap()[yrow : yrow + 128, h0:h1],
                            in_=ya[:, h0:h1],
                        )

    nc.compile()
    return nc


def _host_route(type_embeddings, gate_w):
    """Gate on host: per-type top-2 experts + softmax weights (tiny)."""
    logits = type_embeddings.astype(np.float32) @ gate_w.astype(np.float32)
    top2 = np.argsort(-logits, axis=1, kind="stable")[:, :2]
    tv = np.take_along_axis(logits, top2, axis=1)
    e = np.exp(tv - tv.max(axis=1, keepdims=True))
    wts = e / e.sum(axis=1, keepdims=True)
    return top2, wts


def _xt_layout(buf):
    """[nslots, slen, 256] fp32 -> [nslots*128, 2*slen] bf16 with
    row = slot*128 + p, col = k*slen + a, value = buf[slot, a, k*128+p]."""
    ns, slen, _ = buf.shape
    return np.ascontiguousarray(
        buf.reshape(ns, slen, 2, 128).transpose(0, 3, 2, 1)  # [slot, p, k, a]
    ).reshape(ns * 128, 2 * slen).astype(BF16)


def kernel(x, type_embeddings, atom_types, gate_w, expert_w, expert_b,
           shared_w, shared_b, _trace=False, _trace_kwargs=None):
    x = np.asarray(x, dtype=np.float32)
    type_embeddings = np.asarray(type_embeddings, dtype=np.float32)
    atom_types = np.asarray(atom_types)
    gate_w = np.asarray(gate_w, dtype=np.float32)
    expert_w = np.asarray(expert_w, dtype=np.float32)
    expert_b = np.asarray(expert_b, dtype=np.float32)
    shared_w = np.asarray(shared_w, dtype=np.float32)
    shared_b = np.asarray(shared_b, dtype=np.float32)

    top2, wts = _host_route(type_embeddings, gate_w)

    flat_t = atom_types.reshape(-1).astype(np.int64)
    N = flat_t.size
    order = np.argsort(flat_t, kind="stable")
    counts = np.bincount(flat_t, minlength=NTYPES)
    starts = np.zeros(NTYPES + 1, np.int64)
    starts[1:] = np.cumsum(counts)
    xs = x.reshape(N, DIN)[order]

    # device computes CAP atoms per type; the tiny overflow (~1% of atoms)
    # is computed on the host in fp32, exactly like the reference
    TPC = NTYPES // N_CORES  # big slots per core = 4

    big_buf = np.zeros((N_CORES, TPC, CAP, DIN), np.float32)
    for t in range(NTYPES):
        m = min(int(counts[t]), CAP)
        big_buf[t // TPC, t % TPC, :m] = xs[starts[t] : starts[t] + m]

    spill_out = {}
    for t in range(NTYPES):
        if counts[t] > CAP:
            seg = xs[starts[t] + CAP : starts[t + 1]]
            e0, e1 = top2[t]
            z0 = np.tanh(seg @ expert_w[e0] + expert_b[e0])
            z1 = np.tanh(seg @ expert_w[e1] + expert_b[e1])
            zs = np.tanh(seg @ shared_w[0] + shared_b[0])
            spill_out[t] = wts[t, 0] * z0 + wts[t, 1] * z1 + zs

    in_maps = []
    for core in range(N_CORES):
        sts = [core * TPC + g for g in range(TPC)]
        nslots = len(sts)
        w_sel = np.empty((nslots, NS, DIN, DOUT), np.float32)
        b_sel = np.empty((nslots, NS, DOUT), np.float32)
        s_sel = np.empty((nslots, 2), np.float32)
        for i, t in enumerate(sts):
            e0, e1 = top2[t]
            w_sel[i, 0], w_sel[i, 1], w_sel[i, 2] = (
                expert_w[e0], expert_w[e1], shared_w[0],
            )
            b_sel[i, 0], b_sel[i, 1], b_sel[i, 2] = (
                expert_b[e0], expert_b[e1], shared_b[0],
            )
            s_sel[i] = wts[t]

        wb = (
            w_sel.reshape(nslots, NS, 2, 128, 2, 128)  # [i, s, k, p, c, q]
            .transpose(3, 0, 1, 4, 2, 5)  # [p, i, s, c, k, q]
            .reshape(128, nslots * WCOL)
            .astype(BF16)
        )
        bb = (
            b_sel.reshape(nslots, NS, 2, 128)  # [i, s, c, p]
            .transpose(3, 0, 1, 2)
            .reshape(128, nslots * NS * 2)
            .astype(np.float32)
        )
        sb_arr = np.broadcast_to(
            s_sel.reshape(1, nslots * 2), (128, nslots * 2)
        ).astype(np.float32)

        im = {
            "xtb": _xt_layout(big_buf[core]),
            "w": np.ascontiguousarray(wb),
            "b": np.ascontiguousarray(bb),
            "s": np.ascontiguousarray(sb_arr),
        }
        in_maps.append(im)

    key = (TPC, 0, 0)
    if key not in _compiled_cache:
        _compiled_cache[key] = _build_nc(TPC, 0, 0)
    nc = _compiled_cache[key]

    kwargs = {}
    if _trace:
        kwargs["trace"] = True
        kwargs.update(_trace_kwargs or {})
    res = run_bass_kernel_spmd(nc, in_maps, core_ids=list(range(N_CORES)), **kwargs)

    # reassemble
    out_sorted = np.empty((N, DOUT), np.float32)
    for core in range(N_CORES):
        yb = res.results[core]["yb"].astype(np.float32).reshape(TPC, 2, 128, CAP)
        for g in range(TPC):
            t = core * TPC + g
            m = min(int(counts[t]), CAP)
            # [c, p, a] -> [a, c*128+p]
            blk = yb[g, :, :, :m].reshape(256, m).T
            out_sorted[starts[t] : starts[t] + m] = blk
    for t, seg_out in spill_out.items():
        out_sorted[starts[t] + CAP : starts[t + 1]] = seg_out
    out = np.zeros((N, DOUT), np.float32)
    out[order] = out_sorted
    out = out.reshape(NB, NLOC, DOUT)

    if _trace:
        return out, res
    return out
